# revision 12
# baseline (speedup 1.0000x reference)
"""Multi-head attention (B=2, M=N=2048, D=1024, H=16, DH=64) on 8 TRN2 cores.

Sharding: data-parallel over batch (cores 0-3 = batch 0, 4-7 = batch 1),
tensor-parallel over heads within each batch group (4 heads/core).
Per core:
  - PE-transpose queries/keys/values (chan on partitions)
  - f32r projections: Q^T, K^T (64-chan head rows on partitions, pair-packed
    into two 128-partition tiles), V natural (seq on partitions) with an
    appended ones column (row-sum trick)
  - attention per (head, m-chunk): S^T = K h Q_h^T per 128-row n-tile ->
    exp (ScalarE, f32r out) -> O^T accumulation in PSUM via lhsT=[V|1]
    (row 64 = softmax denominators); normalize with reciprocal broadcast
  - AllGather O^T shards (4-core groups) via internal DRAM bounce
  - output projection: each core computes a 256-wide output-channel slice
    (out^T layout) using its host-sliced Wo^T columns; bias folded in.
Host folds bv/bo into one effective bias (bo + Wo @ bv) and drops bk
(row-constant logit shifts cancel in softmax).
"""

import numpy as np

B, M, NSEQ, D = 2, 2048, 2048, 1024
H, DH = 16, 64
HC = 4                # heads per core
PC = HC * DH          # 256 projected channels per core
CT = D // 128         # 8 contraction tiles
NT = NSEQ // 128      # 16 n-tiles
MT = M // 512         # 4 m-chunks
NCORES = 8

_CACHE = {}


def _bcast_rows(ap, n, bass):
    """AP that replicates a single-partition row across n partitions."""
    return bass.AP(
        tensor=ap.tensor,
        offset=ap.offset,
        ap=[[0, n]] + [list(x) for x in ap.ap[1:]],
    )


def _build(single_core=False):
    import concourse.bass as bass
    import concourse.tile as tile
    from concourse import bacc, mybir
    from concourse.masks import make_identity

    F32 = mybir.dt.float32
    F32R = mybir.dt.float32r
    AF = mybir.ActivationFunctionType

    nc = bacc.Bacc(
        "TRN2",
        target_bir_lowering=False,
        debug=False,
        num_devices=1 if single_core else 8,
    )

    xq_d = nc.dram_tensor("xq", [M, D], F32, kind="ExternalInput")
    xk_d = nc.dram_tensor("xk", [NSEQ, D], F32, kind="ExternalInput")
    xv_d = nc.dram_tensor("xv", [NSEQ, D], F32, kind="ExternalInput")
    wqT_d = nc.dram_tensor("wqT", [D, PC], F32, kind="ExternalInput")
    wkT_d = nc.dram_tensor("wkT", [D, PC], F32, kind="ExternalInput")
    wvT_d = nc.dram_tensor("wvT", [D, PC], F32, kind="ExternalInput")
    woT_d = nc.dram_tensor("woT", [D, PC], F32, kind="ExternalInput")
    bq_d = nc.dram_tensor("bq", [PC, 1], F32, kind="ExternalInput")
    bo_d = nc.dram_tensor("bo", [PC, 1], F32, kind="ExternalInput")
    outT_d = nc.dram_tensor("outT", [PC, M], F32, kind="ExternalOutput")
    import os
    debug = bool(int(os.environ.get("KERNEL_DEBUG", "0")))
    if debug:
        qT_dbg = nc.dram_tensor("qT_dbg", [128, 2, M], F32, kind="ExternalOutput")
        kT_dbg = nc.dram_tensor("kT_dbg", [128, 2, NSEQ], F32, kind="ExternalOutput")
        v_dbg = nc.dram_tensor(
            "v_dbg", [128, HC, NT, 128], F32, kind="ExternalOutput"
        )
        agin_dbg = nc.dram_tensor("agin_dbg", [PC, M], F32, kind="ExternalOutput")
        agout_dbg = nc.dram_tensor(
            "agout_dbg", [4 * PC, M], F32, kind="ExternalOutput"
        )

    with tile.TileContext(nc) as tc:
        with (
            tc.tile_pool(name="singles", bufs=1) as singles,
            tc.tile_pool(name="proj_out", bufs=1) as proj_out,
            tc.tile_pool(name="dram", bufs=1, space="DRAM") as dram,
        ):
            ident = singles.tile([128, 128], F32)
            make_identity(nc, ident)
            bq_sb = singles.tile([128, 2], F32)
            nc.sync.dma_start(
                out=bq_sb, in_=bq_d[:, :].rearrange("(o p) w -> p (o w)", p=128)
            )
            bo_sb = singles.tile([128, 2], F32)
            nc.sync.dma_start(
                out=bo_sb, in_=bo_d[:, :].rearrange("(o p) w -> p (o w)", p=128)
            )

            # persistent projected tensors
            qT = proj_out.tile([128, 2, M], F32R)          # [part, pair, m]
            kT = proj_out.tile([128, 2, NSEQ], F32R)
            # lhsT layout for the AV matmul: col 0 = ones (softmax denom ->
            # PSUM partition 0), cols 1..63 zero, cols 64..127 = V rows
            # (-> PSUM partitions 64..127; DVE 64-partition accesses must
            # start at 0 or 64).
            VE = 128
            v_ext = proj_out.tile([128, HC, NT, VE], F32R)
            with tc.tile_pool(name="vinit", bufs=1) as vinit:
                ones_sb = vinit.tile([128, HC * NT], F32)
                nc.vector.memset(ones_sb, 1.0)
                nc.vector.tensor_copy(
                    v_ext[:, :, :, 0:1],
                    ones_sb[:, :].rearrange("p (h n w) -> p h n w", h=HC, w=1),
                )
                zeros_sb = vinit.tile([128, NT * 63], F32)
                nc.vector.memset(zeros_sb, 0.0)
                for h in range(HC):
                    nc.vector.tensor_copy(
                        v_ext[:, h, :, 1:64],
                        zeros_sb[:, :].rearrange("p (n w) -> p n w", w=63),
                    )

            ag_in = dram.tile([PC, M], F32R)
            ag_out = dram.tile([4 * PC, M], F32R)

            # ---------------- stage 1: transpose + project ----------------
            with (
                tc.tile_pool(name="xT", bufs=1) as xT_pool,
                tc.tile_pool(name="nat", bufs=6) as nat_pool,
                tc.tile_pool(name="wsb", bufs=2) as w_pool,
                tc.tile_pool(name="ps_tr", bufs=4, space="PSUM") as ps_tr,
                tc.tile_pool(name="ps_pj", bufs=4, space="PSUM") as ps_pj,
            ):
                for ti, (x_d, w_d) in enumerate(
                    [(xq_d, wqT_d), (xk_d, wkT_d), (xv_d, wvT_d)]
                ):
                    w_sb = w_pool.tile([128, CT, PC], F32R, tag="w", name=f"w{ti}")
                    nc.gpsimd.dma_start(
                        out=w_sb,
                        in_=w_d[:, :].rearrange("(ct p) c -> p ct c", p=128),
                    )
                    xT = xT_pool.tile([128, CT, M], F32R, tag="xT")
                    # transpose x into xT
                    for rtg in range(4):
                        nats = []
                        for i in range(4):
                            nt_t = nat_pool.tile([128, D], F32, tag="nat", name=f"nat{ti}_{rtg}_{i}")
                            r0 = (rtg * 4 + i) * 128
                            nc.sync.dma_start(out=nt_t, in_=x_d[r0 : r0 + 128, :])
                            nats.append(nt_t)
                        for ct in range(CT):
                            pst = ps_tr.tile([128, 512], F32, tag="pst", name=f"pst{ti}_{rtg}_{ct}")
                            for i in range(4):
                                nc.tensor.transpose(
                                    pst[:, i * 128 : (i + 1) * 128],
                                    nats[i][:, ct * 128 : (ct + 1) * 128],
                                    ident,
                                )
                            dst = xT[:, ct, rtg * 512 : (rtg + 1) * 512]
                            if ct % 2 == 0:
                                nc.vector.tensor_copy(dst, pst)
                            else:
                                nc.scalar.activation(dst, pst, AF.Copy)
                    # project
                    if ti < 2:  # Q^T / K^T: [oc on partitions, m free]
                        dst_T = qT if ti == 0 else kT
                        for ot in range(2):
                            for mh in range(2):
                                pss = [
                                    ps_pj.tile(
                                        [128, 512], F32, tag="pspj",
                                        name=f"pspj{ti}_{ot}_{mh}_{k}",
                                    )
                                    for k in range(2)
                                ]
                                for ct in range(CT):
                                    for mi in range(2):
                                        m = mh * 2 + mi
                                        nc.tensor.matmul(
                                            pss[mi],
                                            w_sb[:, ct, ot * 128 : (ot + 1) * 128],
                                            xT[:, ct, m * 512 : (m + 1) * 512],
                                            start=(ct == 0),
                                            stop=(ct == CT - 1),
                                        )
                                for mi in range(2):
                                    m = mh * 2 + mi
                                    dst = dst_T[:, ot, m * 512 : (m + 1) * 512]
                                    if ti == 0:
                                        nc.scalar.activation(
                                            dst, pss[mi], AF.Identity,
                                            bias=bq_sb[:, ot : ot + 1],
                                        )
                                    else:
                                        # bk dropped: row-constant logit shift
                                        # cancels in softmax
                                        nc.scalar.activation(dst, pss[mi], AF.Copy)
                    else:  # V natural: [n on partitions, head chans free]
                        for nt in range(NT):
                            psv = ps_pj.tile([128, 512], F32, tag="pspj", name=f"psv{nt}")
                            for ct in range(CT):
                                nc.tensor.matmul(
                                    psv[:, 0:PC],
                                    xT[:, ct, nt * 128 : (nt + 1) * 128],
                                    w_sb[:, ct, :],
                                    start=(ct == 0),
                                    stop=(ct == CT - 1),
                                )
                            nc.vector.tensor_copy(
                                v_ext[:, :, nt, 64 : 64 + DH],
                                psv[:, 0:PC].rearrange("p (h d) -> p h d", h=HC),
                            )

            # ---------------- stage 2: attention ----------------
            with (
                tc.tile_pool(name="ps_qk", bufs=2, space="PSUM") as ps_qk,
                tc.tile_pool(name="ps_av", bufs=2, space="PSUM") as ps_av,
                tc.tile_pool(name="at", bufs=3) as at_pool,
                tc.tile_pool(name="eps", bufs=3) as eps_pool,
            ):
                for h in range(HC):
                    p, base = h // 2, (h % 2) * 64
                    for m in range(MT):
                        ps_o = ps_av.tile([128, 512], F32, tag="pso", name=f"pso{h}_{m}")
                        for ng in range(NT // 2):
                            ps_s = ps_qk.tile([128, 1024], F32, tag="pss", name=f"pss{h}_{m}_{ng}")
                            for j in range(2):
                                nt = ng * 2 + j
                                nc.tensor.matmul(
                                    ps_s[:, j * 512 : (j + 1) * 512],
                                    kT[base : base + 64, p, nt * 128 : (nt + 1) * 128],
                                    qT[base : base + 64, p, m * 512 : (m + 1) * 512],
                                    start=True,
                                    stop=True,
                                )
                            at = at_pool.tile([128, 1024], F32R, tag="at", name=f"at{h}_{m}_{ng}")
                            nc.scalar.activation(at, ps_s, AF.Exp)
                            for j in range(2):
                                nt = ng * 2 + j
                                nc.tensor.matmul(
                                    ps_o,
                                    v_ext[:, h, nt, :],
                                    at[:, j * 512 : (j + 1) * 512],
                                    start=(ng == 0 and j == 0),
                                    stop=(ng == NT // 2 - 1 and j == 1),
                                )
                        rec = eps_pool.tile([1, 512], F32, tag="rec", name=f"rec{h}_{m}")
                        nc.vector.reciprocal(rec, ps_o[0:1, :])
                        rbc = eps_pool.tile([128, 512], F32, tag="rbc", name=f"rbc{h}_{m}")
                        nc.gpsimd.partition_broadcast(rbc, rec[0:1, :])
                        osc = eps_pool.tile([128, 512], F32R, tag="osc", name=f"osc{h}_{m}")
                        nc.vector.tensor_mul(
                            osc[64:128, :], ps_o[64:128, :], rbc[64:128, :]
                        )
                        nc.sync.dma_start(
                            out=ag_in[h * DH : (h + 1) * DH, m * 512 : (m + 1) * 512],
                            in_=osc[64:128, :],
                        )

            # ---------------- stage 3: allgather + output projection ----
            if single_core:
                # stand-in for the AllGather so TimelineSim (single-core,
                # no collectives) can model the rest of the pipeline
                for rr in range(4):
                    nc.sync.dma_start(
                        out=ag_out[rr * PC : (rr + 1) * PC, :], in_=ag_in[:, :]
                    )
            else:
                nc.gpsimd.collective_compute(
                    "AllGather",
                    bass.mybir.AluOpType.bypass,
                    replica_groups=[[0, 1, 2, 3], [4, 5, 6, 7]],
                    ins=[ag_in[:, :].opt()],
                    outs=[ag_out[:, :].opt()],
                )

            if debug:
                nc.sync.dma_start(
                    out=qT_dbg[:, :, :], in_=qT[:, :, :].bitcast(F32)
                )
                nc.sync.dma_start(
                    out=kT_dbg[:, :, :], in_=kT[:, :, :].bitcast(F32)
                )
                nc.sync.dma_start(
                    out=v_dbg[:, :, :, :], in_=v_ext[:, :, :, :].bitcast(F32)
                )
                nc.gpsimd.dma_start(
                    out=agin_dbg[:, :], in_=ag_in[:, :].bitcast(F32)
                )
                nc.gpsimd.dma_start(
                    out=agout_dbg[:, :], in_=ag_out[:, :].bitcast(F32)
                )

            with (
                tc.tile_pool(name="og", bufs=1) as og_pool,
                tc.tile_pool(name="wo", bufs=1) as wo_pool,
                tc.tile_pool(name="osb", bufs=4) as osb_pool,
                tc.tile_pool(name="ps_o2", bufs=3, space="PSUM") as ps_o2,
            ):
                wo_sb = wo_pool.tile([128, CT, PC], F32R)
                nc.gpsimd.dma_start(
                    out=wo_sb,
                    in_=woT_d[:, :].rearrange("(ct p) c -> p ct c", p=128),
                )
                og = og_pool.tile([128, CT, M], F32R)
                for ct in range(CT):
                    nc.sync.dma_start(
                        out=og[:, ct, :], in_=ag_out[ct * 128 : (ct + 1) * 128, :]
                    )
                for ot in range(2):
                    for m in range(MT):
                        po = ps_o2.tile([128, 512], F32, tag="po", name=f"po{ot}_{m}")
                        for ct in range(CT):
                            nc.tensor.matmul(
                                po,
                                wo_sb[:, ct, ot * 128 : (ot + 1) * 128],
                                og[:, ct, m * 512 : (m + 1) * 512],
                                start=(ct == 0),
                                stop=(ct == CT - 1),
                            )
                        osb = osb_pool.tile([128, 512], F32, tag="osb", name=f"osb{ot}_{m}")
                        nc.scalar.activation(
                            osb, po, AF.Identity, bias=bo_sb[:, ot : ot + 1]
                        )
                        nc.sync.dma_start(
                            out=outT_d[
                                ot * 128 : (ot + 1) * 128, m * 512 : (m + 1) * 512
                            ],
                            in_=osb,
                        )
    nc.compile()
    return nc


def kernel(queries, keys, values, Wq, bq, Wk, bk, Wv, bv, Wo, bo, _trace=False):
    import concourse.bass_utils as bass_utils

    queries = np.asarray(queries, dtype=np.float32)
    keys = np.asarray(keys, dtype=np.float32)
    values = np.asarray(values, dtype=np.float32)
    Wq = np.asarray(Wq, dtype=np.float32)
    Wk = np.asarray(Wk, dtype=np.float32)
    Wv = np.asarray(Wv, dtype=np.float32)
    Wo = np.asarray(Wo, dtype=np.float32)
    bq = np.asarray(bq, dtype=np.float32)
    bk = np.asarray(bk, dtype=np.float32)
    bv = np.asarray(bv, dtype=np.float32)
    bo = np.asarray(bo, dtype=np.float32)

    if "nc" not in _CACHE:
        _CACHE["nc"] = _build()
    nc = _CACHE["nc"]

    # bv folds through attention (sum of weights = 1) and the output
    # projection into an effective output bias; bk shifts every logit in a
    # row equally so softmax cancels it.
    bo_eff = bo + Wo @ bv

    c = np.ascontiguousarray
    in_maps = []
    for core in range(NCORES):
        b, r = core // 4, core % 4
        sl = slice(r * PC, (r + 1) * PC)
        in_maps.append(
            {
                "xq": c(queries[b]),
                "xk": c(keys[b]),
                "xv": c(values[b]),
                "wqT": c(Wq[sl, :].T),
                "wkT": c(Wk[sl, :].T),
                "wvT": c(Wv[sl, :].T),
                "woT": c(Wo.T[:, sl]),
                "bq": c(bq[sl].reshape(PC, 1)),
                "bo": c(bo_eff[sl].reshape(PC, 1)),
            }
        )

    res = bass_utils.run_bass_kernel_spmd(
        nc, in_maps, core_ids=list(range(NCORES)), trace=_trace
    )
    _CACHE["last_result"] = res

    out = np.empty((B, M, D), dtype=np.float32)
    for core in range(NCORES):
        b, r = core // 4, core % 4
        out[b, :, r * PC : (r + 1) * PC] = res.results[core]["outT"].T
    return out


# revision 13
# speedup vs baseline: 151.7484x; 151.7484x over previous
"""Multi-head attention (B=2, M=N=2048, D=1024, H=16, DH=64) on 8 TRN2 cores.

Sharding: data-parallel over batch (cores 0-3 = batch 0, 4-7 = batch 1),
tensor-parallel over heads within each batch group (4 heads/core).
Per core:
  - PE-transpose queries/keys/values (chan on partitions)
  - f32r projections: Q^T, K^T (64-chan head rows on partitions, pair-packed
    into two 128-partition tiles), V natural (seq on partitions) in an
    extended lhsT layout [ones | zeros | V] (softmax row-sum trick)
  - attention per (head, m-chunk): S^T = K_h Q_h^T per 128-row n-tile ->
    exp (ScalarE, f32r out) -> O^T accumulation in PSUM via lhsT=[1|0|V]
    (partition 0 = softmax denominators, partitions 64..127 = O^T rows);
    normalize with reciprocal + gpsimd partition-broadcast
  - AllGather O^T shards (4-core groups) via internal DRAM bounce
  - output projection: each core computes a 256-wide output-channel slice
    (out^T layout) using its host-sliced Wo^T columns; bias folded in.
Host folds bv/bo into one effective bias (bo + Wo @ bv) and drops bk
(row-constant logit shifts cancel in softmax).
"""

import os

import numpy as np

B, M, NSEQ, D = 2, 2048, 2048, 1024
H, DH = 16, 64
HC = 4                # heads per core
PC = HC * DH          # 256 projected channels per core
CT = D // 128         # 8 contraction tiles
NT = NSEQ // 128      # 16 n-tiles
MT = M // 512         # 4 m-chunks
NCORES = 8

_CACHE = {}


def _build(single_core=False, reps=1):
    import concourse.bass as bass
    import concourse.tile as tile
    from concourse import bacc, mybir
    from concourse.masks import make_identity

    F32 = mybir.dt.float32
    F32R = mybir.dt.float32r
    AF = mybir.ActivationFunctionType

    nc = bacc.Bacc(
        "TRN2",
        target_bir_lowering=False,
        debug=False,
        num_devices=1 if single_core else 8,
    )

    xq_d = nc.dram_tensor("xq", [M, D], F32, kind="ExternalInput")
    xk_d = nc.dram_tensor("xk", [NSEQ, D], F32, kind="ExternalInput")
    xv_d = nc.dram_tensor("xv", [NSEQ, D], F32, kind="ExternalInput")
    wqT_d = nc.dram_tensor("wqT", [D, PC], F32, kind="ExternalInput")
    wkT_d = nc.dram_tensor("wkT", [D, PC], F32, kind="ExternalInput")
    wvT_d = nc.dram_tensor("wvT", [D, PC], F32, kind="ExternalInput")
    woT_d = nc.dram_tensor("woT", [D, PC], F32, kind="ExternalInput")
    bq_d = nc.dram_tensor("bq", [PC, 1], F32, kind="ExternalInput")
    bo_d = nc.dram_tensor("bo", [PC, 1], F32, kind="ExternalInput")
    outT_d = nc.dram_tensor("outT", [PC, M], F32, kind="ExternalOutput")
    debug = bool(int(os.environ.get("KERNEL_DEBUG", "0")))
    if debug:
        qT_dbg = nc.dram_tensor("qT_dbg", [128, 2, M], F32, kind="ExternalOutput")
        kT_dbg = nc.dram_tensor("kT_dbg", [128, 2, NSEQ], F32, kind="ExternalOutput")
        v_dbg = nc.dram_tensor(
            "v_dbg", [128, HC, NT, 128], F32, kind="ExternalOutput"
        )
        agin_dbg = nc.dram_tensor("agin_dbg", [PC, M], F32, kind="ExternalOutput")
        agout_dbg = nc.dram_tensor(
            "agout_dbg", [4 * PC, M], F32, kind="ExternalOutput"
        )

    with tile.TileContext(nc) as tc:
        with (
            tc.tile_pool(name="singles", bufs=1) as singles,
            tc.tile_pool(name="dram", bufs=1, space="DRAM") as dram,
        ):
            ident = singles.tile([128, 128], F32)
            make_identity(nc, ident)
            bq_sb = singles.tile([128, 2], F32)
            nc.sync.dma_start(
                out=bq_sb, in_=bq_d[:, :].rearrange("(o p) w -> p (o w)", p=128)
            )
            bo_sb = singles.tile([128, 2], F32)
            nc.sync.dma_start(
                out=bo_sb, in_=bo_d[:, :].rearrange("(o p) w -> p (o w)", p=128)
            )

            ag_in = dram.tile([PC, M], F32R)
            ag_out = dram.tile([4 * PC, M], F32R)

            for rep in range(reps):
                _emit_rep(
                    nc, tc, bass, mybir, F32, F32R, AF, rep, single_core,
                    debug and rep == reps - 1,
                    dict(
                        xq_d=xq_d, xk_d=xk_d, xv_d=xv_d, wqT_d=wqT_d,
                        wkT_d=wkT_d, wvT_d=wvT_d, woT_d=woT_d, outT_d=outT_d,
                        ident=ident, bq_sb=bq_sb, bo_sb=bo_sb,
                        ag_in=ag_in, ag_out=ag_out,
                        dbg=dict(
                            qT_dbg=qT_dbg, kT_dbg=kT_dbg, v_dbg=v_dbg,
                            agin_dbg=agin_dbg, agout_dbg=agout_dbg,
                        ) if debug else None,
                    ),
                )
    nc.compile()
    return nc


def _emit_rep(nc, tc, bass, mybir, F32, F32R, AF, rep, single_core, debug, env):
    ident = env["ident"]
    bq_sb, bo_sb = env["bq_sb"], env["bo_sb"]
    ag_in, ag_out = env["ag_in"], env["ag_out"]
    R = f"r{rep}_"

    with tc.tile_pool(name=f"{R}proj_out", bufs=1) as proj_out:
        qT = proj_out.tile([128, 2, M], F32R, name=f"{R}qT")  # [part, pair, m]
        kT = proj_out.tile([128, 2, NSEQ], F32R, name=f"{R}kT")
        # lhsT layout for the AV matmul: col 0 = ones (softmax denom ->
        # PSUM partition 0), cols 1..63 zero, cols 64..127 = V rows
        # (-> PSUM partitions 64..127; DVE 64-partition accesses must
        # start at 0 or 64).
        v_ext = proj_out.tile([128, HC, NT, 128], F32R, name=f"{R}v_ext")
        with tc.tile_pool(name=f"{R}vinit", bufs=1) as vinit:
            ones_sb = vinit.tile([128, HC * NT], F32, name=f"{R}ones")
            nc.vector.memset(ones_sb, 1.0)
            nc.vector.tensor_copy(
                v_ext[:, :, :, 0:1],
                ones_sb[:, :].rearrange("p (h n w) -> p h n w", h=HC, w=1),
            )
            zeros_sb = vinit.tile([128, NT * 63], F32, name=f"{R}zeros")
            nc.vector.memset(zeros_sb, 0.0)
            for h in range(HC):
                nc.vector.tensor_copy(
                    v_ext[:, h, :, 1:64],
                    zeros_sb[:, :].rearrange("p (n w) -> p n w", w=63),
                )

        # ---------------- stage 1: transpose + project ----------------
        with (
            tc.tile_pool(name=f"{R}xT", bufs=1) as xT_pool,
            tc.tile_pool(name=f"{R}nat", bufs=6) as nat_pool,
            tc.tile_pool(name=f"{R}wsb", bufs=2) as w_pool,
            tc.tile_pool(name=f"{R}ps_tr", bufs=4, space="PSUM") as ps_tr,
            tc.tile_pool(name=f"{R}ps_pj", bufs=4, space="PSUM") as ps_pj,
        ):
            for ti, (x_d, w_d) in enumerate(
                [
                    (env["xq_d"], env["wqT_d"]),
                    (env["xk_d"], env["wkT_d"]),
                    (env["xv_d"], env["wvT_d"]),
                ]
            ):
                w_sb = w_pool.tile(
                    [128, CT, PC], F32R, tag="w", name=f"{R}w{ti}"
                )
                nc.gpsimd.dma_start(
                    out=w_sb,
                    in_=w_d[:, :].rearrange("(ct p) c -> p ct c", p=128),
                )
                xT = xT_pool.tile([128, CT, M], F32R, tag="xT", name=f"{R}xT{ti}")
                # transpose x into xT
                for rtg in range(4):
                    nats = []
                    for i in range(4):
                        nt_t = nat_pool.tile(
                            [128, D], F32, tag="nat", name=f"{R}nat{ti}_{rtg}_{i}"
                        )
                        r0 = (rtg * 4 + i) * 128
                        nc.sync.dma_start(out=nt_t, in_=x_d[r0 : r0 + 128, :])
                        nats.append(nt_t)
                    for ct in range(CT):
                        pst = ps_tr.tile(
                            [128, 512], F32, tag="pst", name=f"{R}pst{ti}_{rtg}_{ct}"
                        )
                        for i in range(4):
                            nc.tensor.transpose(
                                pst[:, i * 128 : (i + 1) * 128],
                                nats[i][:, ct * 128 : (ct + 1) * 128],
                                ident,
                            )
                        dst = xT[:, ct, rtg * 512 : (rtg + 1) * 512]
                        if ct % 2 == 0:
                            nc.vector.tensor_copy(dst, pst)
                        else:
                            nc.scalar.activation(dst, pst, AF.Copy)
                # project
                if ti < 2:  # Q^T / K^T: [oc on partitions, m free]
                    dst_T = qT if ti == 0 else kT
                    for ot in range(2):
                        for mh in range(2):
                            pss = [
                                ps_pj.tile(
                                    [128, 512], F32, tag="pspj",
                                    name=f"{R}pspj{ti}_{ot}_{mh}_{k}",
                                )
                                for k in range(2)
                            ]
                            for ct in range(CT):
                                for mi in range(2):
                                    m = mh * 2 + mi
                                    nc.tensor.matmul(
                                        pss[mi],
                                        w_sb[:, ct, ot * 128 : (ot + 1) * 128],
                                        xT[:, ct, m * 512 : (m + 1) * 512],
                                        start=(ct == 0),
                                        stop=(ct == CT - 1),
                                    )
                            for mi in range(2):
                                m = mh * 2 + mi
                                dst = dst_T[:, ot, m * 512 : (m + 1) * 512]
                                if ti == 0:
                                    nc.scalar.activation(
                                        dst, pss[mi], AF.Identity,
                                        bias=bq_sb[:, ot : ot + 1],
                                    )
                                else:
                                    # bk dropped: row-constant logit shift
                                    # cancels in softmax
                                    nc.scalar.activation(dst, pss[mi], AF.Copy)
                else:  # V natural: [n on partitions, head chans free]
                    for nt in range(NT):
                        psv = ps_pj.tile(
                            [128, 512], F32, tag="pspj", name=f"{R}psv{nt}"
                        )
                        for ct in range(CT):
                            nc.tensor.matmul(
                                psv[:, 0:PC],
                                xT[:, ct, nt * 128 : (nt + 1) * 128],
                                w_sb[:, ct, :],
                                start=(ct == 0),
                                stop=(ct == CT - 1),
                            )
                        nc.vector.tensor_copy(
                            v_ext[:, :, nt, 64 : 64 + DH],
                            psv[:, 0:PC].rearrange("p (h d) -> p h d", h=HC),
                        )

        # ---------------- stage 2: attention ----------------
        with (
            tc.tile_pool(name=f"{R}ps_qk", bufs=2, space="PSUM") as ps_qk,
            tc.tile_pool(name=f"{R}ps_av", bufs=2, space="PSUM") as ps_av,
            tc.tile_pool(name=f"{R}at", bufs=3) as at_pool,
            tc.tile_pool(name=f"{R}eps", bufs=3) as eps_pool,
        ):
            for h in range(HC):
                p, base = h // 2, (h % 2) * 64
                for m in range(MT):
                    ps_o = ps_av.tile(
                        [128, 512], F32, tag="pso", name=f"{R}pso{h}_{m}"
                    )
                    for ng in range(NT // 2):
                        ps_s = ps_qk.tile(
                            [128, 1024], F32, tag="pss", name=f"{R}pss{h}_{m}_{ng}"
                        )
                        for j in range(2):
                            nt = ng * 2 + j
                            nc.tensor.matmul(
                                ps_s[:, j * 512 : (j + 1) * 512],
                                kT[base : base + 64, p, nt * 128 : (nt + 1) * 128],
                                qT[base : base + 64, p, m * 512 : (m + 1) * 512],
                                start=True,
                                stop=True,
                            )
                        at = at_pool.tile(
                            [128, 1024], F32R, tag="at", name=f"{R}at{h}_{m}_{ng}"
                        )
                        nc.scalar.activation(at, ps_s, AF.Exp)
                        for j in range(2):
                            nt = ng * 2 + j
                            nc.tensor.matmul(
                                ps_o,
                                v_ext[:, h, nt, :],
                                at[:, j * 512 : (j + 1) * 512],
                                start=(ng == 0 and j == 0),
                                stop=(ng == NT // 2 - 1 and j == 1),
                            )
                    rec = eps_pool.tile([1, 512], F32, tag="rec", name=f"{R}rec{h}_{m}")
                    nc.vector.reciprocal(rec, ps_o[0:1, :])
                    rbc = eps_pool.tile(
                        [128, 512], F32, tag="rbc", name=f"{R}rbc{h}_{m}"
                    )
                    nc.gpsimd.partition_broadcast(rbc, rec[0:1, :])
                    osc = eps_pool.tile(
                        [128, 512], F32R, tag="osc", name=f"{R}osc{h}_{m}"
                    )
                    nc.vector.tensor_mul(
                        osc[64:128, :], ps_o[64:128, :], rbc[64:128, :]
                    )
                    nc.sync.dma_start(
                        out=ag_in[h * DH : (h + 1) * DH, m * 512 : (m + 1) * 512],
                        in_=osc[64:128, :],
                    )

        # ---------------- stage 3: allgather + output projection ----
        if single_core:
            # stand-in for the AllGather so TimelineSim (single-core,
            # no collectives) can model the rest of the pipeline
            for rr in range(4):
                nc.sync.dma_start(
                    out=ag_out[rr * PC : (rr + 1) * PC, :], in_=ag_in[:, :]
                )
        else:
            nc.gpsimd.collective_compute(
                "AllGather",
                bass.mybir.AluOpType.bypass,
                replica_groups=[[0, 1, 2, 3], [4, 5, 6, 7]],
                ins=[ag_in[:, :].opt()],
                outs=[ag_out[:, :].opt()],
            )

        if debug:
            d = env["dbg"]
            nc.sync.dma_start(out=d["qT_dbg"][:, :, :], in_=qT[:, :, :].bitcast(F32))
            nc.sync.dma_start(out=d["kT_dbg"][:, :, :], in_=kT[:, :, :].bitcast(F32))
            nc.sync.dma_start(
                out=d["v_dbg"][:, :, :, :], in_=v_ext[:, :, :, :].bitcast(F32)
            )
            nc.gpsimd.dma_start(
                out=d["agin_dbg"][:, :], in_=ag_in[:, :].bitcast(F32)
            )
            nc.gpsimd.dma_start(
                out=d["agout_dbg"][:, :], in_=ag_out[:, :].bitcast(F32)
            )

        with (
            tc.tile_pool(name=f"{R}og", bufs=1) as og_pool,
            tc.tile_pool(name=f"{R}wo", bufs=1) as wo_pool,
            tc.tile_pool(name=f"{R}osb", bufs=4) as osb_pool,
            tc.tile_pool(name=f"{R}ps_o2", bufs=3, space="PSUM") as ps_o2,
        ):
            wo_sb = wo_pool.tile([128, CT, PC], F32R, name=f"{R}wo_sb")
            nc.gpsimd.dma_start(
                out=wo_sb,
                in_=env["woT_d"][:, :].rearrange("(ct p) c -> p ct c", p=128),
            )
            og = og_pool.tile([128, CT, M], F32R, name=f"{R}og")
            for ct in range(CT):
                nc.sync.dma_start(
                    out=og[:, ct, :], in_=ag_out[ct * 128 : (ct + 1) * 128, :]
                )
            for ot in range(2):
                for m in range(MT):
                    po = ps_o2.tile(
                        [128, 512], F32, tag="po", name=f"{R}po{ot}_{m}"
                    )
                    for ct in range(CT):
                        nc.tensor.matmul(
                            po,
                            wo_sb[:, ct, ot * 128 : (ot + 1) * 128],
                            og[:, ct, m * 512 : (m + 1) * 512],
                            start=(ct == 0),
                            stop=(ct == CT - 1),
                        )
                    osb = osb_pool.tile(
                        [128, 512], F32, tag="osb", name=f"{R}osb{ot}_{m}"
                    )
                    nc.scalar.activation(
                        osb, po, AF.Identity, bias=bo_sb[:, ot : ot + 1]
                    )
                    nc.sync.dma_start(
                        out=env["outT_d"][
                            ot * 128 : (ot + 1) * 128, m * 512 : (m + 1) * 512
                        ],
                        in_=osb,
                    )


def _make_in_maps(queries, keys, values, Wq, bq, Wk, bk, Wv, bv, Wo, bo):
    # bv folds through attention (softmax weights sum to 1) and the output
    # projection into an effective output bias; bk shifts every logit in a
    # row equally so softmax cancels it.
    bo_eff = bo + Wo @ bv
    c = np.ascontiguousarray
    in_maps = []
    for core in range(NCORES):
        b, r = core // 4, core % 4
        sl = slice(r * PC, (r + 1) * PC)
        in_maps.append(
            {
                "xq": c(queries[b]),
                "xk": c(keys[b]),
                "xv": c(values[b]),
                "wqT": c(Wq[sl, :].T),
                "wkT": c(Wk[sl, :].T),
                "wvT": c(Wv[sl, :].T),
                "woT": c(Wo.T[:, sl]),
                "bq": c(bq[sl].reshape(PC, 1)),
                "bo": c(bo_eff[sl].reshape(PC, 1)),
            }
        )
    return in_maps


def kernel(queries, keys, values, Wq, bq, Wk, bk, Wv, bv, Wo, bo, _trace=False):
    import concourse.bass_utils as bass_utils

    args = [queries, keys, values, Wq, bq, Wk, bk, Wv, bv, Wo, bo]
    args = [np.asarray(a, dtype=np.float32) for a in args]

    if "nc" not in _CACHE:
        _CACHE["nc"] = _build()
    nc = _CACHE["nc"]

    in_maps = _make_in_maps(*args)
    res = bass_utils.run_bass_kernel_spmd(
        nc, in_maps, core_ids=list(range(NCORES)), trace=_trace
    )
    _CACHE["last_result"] = res

    out = np.empty((B, M, D), dtype=np.float32)
    for core in range(NCORES):
        b, r = core // 4, core % 4
        out[b, :, r * PC : (r + 1) * PC] = res.results[core]["outT"].T
    return out
